# revision 50
# baseline (speedup 1.0000x reference)
"""Trainium2 Bass kernel: AttentionBlock (GroupNorm + 1x1-conv QKV + MHA + proj + residual).

Data-parallel over batch: 16 samples -> 8 NeuronCores x 2 samples. Each core
runs the whole block locally (attention is per-sample, no collectives); the
host shards inputs and concatenates the 8 output shards.

Math notes (exact rewrites, not approximations):
  - scores are computed transposed, S^T[m,n] = sum_d k[d,m] q'[d,n] with
    q' = (q + b_q) * d^-0.5. The k-bias adds a column-constant to S^T which
    softmax cancels, so it is dropped.
  - softmax denominator comes from a ones-column appended to v^T in the
    attn@v matmul (row 64 of the [65, n] output accumulates colsum(exp S^T)).
  - v-bias: attn rows sum to 1, so  attn @ (Wv h + bv) = attn @ Wv h + bv;
    the bv term is folded into the proj bias on the host:
    beff = b_proj + w_proj @ bv.
  - groupnorm stats come from per-channel bn_stats on the resident x tiles;
    16-channel group aggregation is a tiny block-diagonal (1/16) matmul that
    simultaneously broadcasts the group stats back to channel lanes.

Approximation (bounded, validated vs the 2e-2 gate at ~7e-4 end-to-end):
  - half the softmax exps (hh=1 of each head pair) use a one-op Schraudolph
    exponential on DVE: f16 bits = round(logit * 1024/ln2 + 15300.7), i.e.
    2^(x/ln2) with a linearly-filled mantissa (max rel err ~3%, mean ~0).
    Softmax renormalization cancels the mean error; the weighted average over
    ~1e2-effective attention positions suppresses the remainder ~30x.

Performance structure (HAM: PE clock is 1.2 GHz until ~3.4us of continuous
activity, 2.4 GHz after; idle windows re-throttle):
  - a warmup chain of dummy matmuls keeps the PE busy from t~0 so the real
    stream starts (and stays) at 2.4 GHz,
  - per-sample stats gating: sample 0's qkv starts as soon as its 2MB of x
    has streamed in (~9us), not after the full batch stats,
  - exp split ACT/DVE makes attention PE-bound instead of ACT-bound,
  - the final drain normalize uses a PE broadcast (f32 selector matmul) of
    the softmax reciprocals instead of the 1us GpSimd partition_broadcast.
"""

import os
import sys
import types
from contextlib import ExitStack

import ml_dtypes
import numpy as np

# If BASS_TRACE is set but this container's antenv lacks the NTFF hook
# module, bass_utils' trace path would crash on import; give it a null
# hook so tracing degrades gracefully instead.
try:
    import antenv.axon_hooks  # noqa: F401
except Exception:  # pragma: no cover
    try:
        import antenv

        _hookmod = types.ModuleType("antenv.axon_hooks")
        _hook = [None]
        _hookmod.set_axon_ntff_profile_hook = lambda h: _hook.__setitem__(0, h)
        _hookmod.get_axon_ntff_profile_hook = lambda: _hook[0]
        sys.modules["antenv.axon_hooks"] = _hookmod
        antenv.axon_hooks = _hookmod
    except Exception:
        pass

import concourse.bass as bass
import concourse.tile as tile
from concourse import bacc
from concourse import mybir
from concourse.bass_utils import run_bass_kernel_spmd

F32 = mybir.dt.float32
F32R = mybir.dt.float32r
BF16 = mybir.dt.bfloat16
F16 = mybir.dt.float16
I16 = mybir.dt.int16
I32 = mybir.dt.int32
AF = mybir.ActivationFunctionType
ALU = mybir.AluOpType

# Problem dims (hardcoded per spec: x [16, 512, 32, 32] f32)
B, C, H, W = 16, 512, 32, 32
N = H * W                # 1024 spatial positions
NCORES = 8
BS = B // NCORES         # 2 samples per core
G = 32                   # groupnorm groups
HEADS = 8
D = C // HEADS           # 64
CT = C // 128            # 4 channel tiles
MT = N // 128            # 8 m-tiles (spatial, attention contraction)
NHALF = 2                # n split in halves of 512 (psum bank limit)
EPS = 1e-5

# Schraudolph f16-bits exp: bits = round(x * 1024/ln2 + (15360 - 59.3))
SCH_K = 1024.0 / float(np.log(2.0))
SCH_C = 15360.0 - 59.3
# exp engine per (pair-half): hh0 exact on ACT, hh1 Schraudolph on DVE
DVE_EXP_HH = {1}
N_WARMUP = 75            # dummy matmuls covering the prologue (~9.5us)

LAST_EXEC_NS = None
LAST_RESULTS = None


def _build_tile(ctx: ExitStack, tc: tile.TileContext, te: dict):
    nc = tc.nc
    x_e, out_e = te["x"], te["out"]

    const = ctx.enter_context(tc.tile_pool(name="const", bufs=1))
    small = ctx.enter_context(tc.tile_pool(name="small", bufs=8))

    # ---- PE warmup: serial dummy matmul chain from t~0 keeps HAM at 8/8 ----
    # (own psum pool, closed before the main psum pools open: 8-bank budget)
    warm_w = const.tile([128, 256], BF16)
    nc.vector.memset(warm_w[:], 0.0)
    with tc.tile_pool(name="warm", bufs=1, space="PSUM") as warmp:
        warm_ps = warmp.tile([128, 256], F32, name="warm_ps")
        for _ in range(N_WARMUP):
            nc.tensor.matmul(
                warm_ps[:], warm_w[:, 0:128], warm_w[:], start=True, stop=True
            )
    ps_acc = ctx.enter_context(tc.tile_pool(name="ps_acc", bufs=4, space="PSUM"))
    ps_sc = ctx.enter_context(tc.tile_pool(name="ps_sc", bufs=2, space="PSUM"))

    # preload the Exp ACT table set off the critical path
    eps_sb = const.tile([1, 1], F32)
    nc.vector.memset(eps_sb[:], EPS)
    dummy_act = const.tile([1, 1], F32)
    nc.scalar.activation(dummy_act[:], eps_sb[:], AF.Exp)

    # ---- x into SBUF (single pass) + weights, spread over 4 DMA queues ----
    # Queue plan (per engine queue, in issue order): sample-0 x tile, then a
    # wqk k-tile, small constants, sample-1 x tile, then wv/wp. Sample 0's
    # stats + first qkv matmuls gate on ~770KB per queue (~9us), not on the
    # whole 6MB.
    xpool = ctx.enter_context(tc.tile_pool(name="xres", bufs=1))
    x_sb = xpool.tile([128, BS * CT, N], BF16)
    stats_sb = const.tile([128, BS * CT, 2, 6], F32)
    wqk_sb = const.tile([128, CT, 2 * C], BF16)   # w_qkv[:1024].T tiles
    wv_sb = const.tile([128, CT, C], BF16)        # w_qkv[1024:].T tiles
    wp_sb = const.tile([128, CT, C], BF16)        # w_proj.T tiles
    bq_sb = const.tile([128, CT, 1], F32)
    beff_sb = const.tile([128, CT, 1], F32)
    gam = const.tile([128, CT], F32)
    bet = const.tile([128, CT], F32)
    selg_sb = const.tile([128, 128], F32R)        # block-diag 1/16 group avg
    dma_eng = [nc.sync, nc.gpsimd, nc.scalar, nc.sync]

    # sample-0 x in 512-col half-chunks, round-robin over the 3 DMA-capable
    # queues, bn_stats right behind each chunk: the stats chain tracks the
    # stream instead of gating on the slowest whole tile.
    qi = 0
    for t in range(CT):
        for hc in range(2):
            dma_eng[qi % 3].dma_start(
                out=x_sb[:, t, hc * 512 : (hc + 1) * 512],
                in_=x_e[0, t * 128 : (t + 1) * 128, hc * 512 : (hc + 1) * 512],
            )
            qi += 1
            nc.vector.bn_stats(
                out=stats_sb[:, t, hc, :],
                in_=x_sb[:, t, hc * 512 : (hc + 1) * 512],
            )
    for kt in range(CT):
        sl = slice(kt * 128, (kt + 1) * 128)
        dma_eng[kt].dma_start(out=wqk_sb[:, kt, :], in_=te["wqkT"][sl, :])
    nc.sync.dma_start(out=selg_sb[:], in_=te["selg"][:])
    nc.sync.dma_start(
        out=gam[:], in_=bass.AP(tensor=te["gamma"], offset=0, ap=[[1, 128], [128, CT]])
    )
    nc.sync.dma_start(
        out=bet[:], in_=bass.AP(tensor=te["beta"], offset=0, ap=[[1, 128], [128, CT]])
    )
    for kt in range(CT):
        sl = slice(kt * 128, (kt + 1) * 128)
        nc.gpsimd.dma_start(out=bq_sb[:, kt, :], in_=te["bq"][sl, :])
    # sample-1 x DMAs issue now (queue order), but their bn_stats are
    # emitted AFTER emit_prep(0): DVE is strict FIFO, and a bn_stats
    # waiting on a late DMA would head-of-line-block sample 0's stat chain.
    for t in range(CT):
        j = CT + t
        dma_eng[t].dma_start(
            out=x_sb[:, j, :], in_=x_e[1, t * 128 : (t + 1) * 128, :]
        )
    for kt in range(CT):
        sl = slice(kt * 128, (kt + 1) * 128)
        dma_eng[kt].dma_start(out=wv_sb[:, kt, :], in_=te["wvT"][sl, :])
    for kt in range(CT):
        sl = slice(kt * 128, (kt + 1) * 128)
        dma_eng[kt].dma_start(out=wp_sb[:, kt, :], in_=te["wpT"][sl, :])
        nc.scalar.dma_start(out=beff_sb[:, kt, :], in_=te["beff"][sl, :])

    # selectors for the final-drain reciprocal broadcast (partition-0 rows;
    # partition-offset writes must be 32-aligned, so two separate tiles)
    sel_lo = const.tile([1, 128], F32)
    nc.vector.memset(sel_lo[:], 0.0)
    nc.vector.memset(sel_lo[0:1, 0:64], 1.0)
    sel_hi = const.tile([1, 128], F32)
    nc.vector.memset(sel_hi[:], 0.0)
    nc.vector.memset(sel_hi[0:1, 64:128], 1.0)

    # ---- per-sample groupnorm stat chain -> A (scale), B (shift) ----
    A_s = [const.tile([128, CT], F32, name=f"A_{s}") for s in range(BS)]
    B_s = [const.tile([128, CT], F32, name=f"B_{s}") for s in range(BS)]

    def emit_stats(s):
        mv = small.tile([128, CT, 2], F32, tag="mv", name="mv")
        for t in range(CT):
            nc.vector.bn_aggr(out=mv[:, t, :], in_=stats_sb[:, s * CT + t, :, :])
        # per-channel (mean, E[x^2]) in f32r for the group-average matmul
        ex2 = small.tile([128, CT], F32, tag="ex2", name="ex2")
        nc.vector.tensor_mul(ex2[:], mv[:, :, 0], mv[:, :, 0])
        nc.vector.tensor_add(ex2[:], ex2[:], mv[:, :, 1])
        pcs = small.tile([128, CT, 2], F32R, tag="pcs", name="pcs")
        nc.vector.tensor_copy(pcs[:, :, 0], mv[:, :, 0])
        nc.vector.tensor_copy(pcs[:, :, 1], ex2[:])
        gst = ps_sc.tile([128, CT, 2], F32, tag="sc", name="gst")
        for t in range(CT):
            nc.tensor.matmul(
                gst[:, t, :], selg_sb[:], pcs[:, t, :], start=True, stop=True
            )
        # gvar = gEx2 - gmean^2 ; rstd via int-seed + 2 Newton steps (DVE only)
        # (gmean bounced to SBUF: DVE has a single PSUM read port)
        gm = small.tile([128, CT], F32, tag="gm", name="gm")
        nc.vector.tensor_copy(gm[:], gst[:, :, 0])
        vpe = small.tile([128, CT], F32, tag="vpe", name="vpe")
        nc.vector.tensor_mul(vpe[:], gm[:], gm[:])
        nc.vector.tensor_sub(vpe[:], gst[:, :, 1], vpe[:])
        nc.vector.tensor_scalar_add(vpe[:], vpe[:], EPS)
        hv = small.tile([128, CT], F32, tag="hv", name="hv")
        nc.vector.tensor_scalar_mul(hv[:], vpe[:], -0.5)
        y0 = small.tile([128, CT], F32, tag="y0", name="y0")
        ysh = small.tile([128, CT], I32, tag="ysh", name="ysh")
        nc.vector.tensor_scalar(
            out=ysh[:], in0=vpe[:].bitcast(I32), scalar1=1, scalar2=None,
            op0=ALU.arith_shift_right,
        )
        nc.vector.tensor_scalar(
            out=y0[:].bitcast(I32), in0=ysh[:], scalar1=-1, scalar2=0x5F3759DF,
            op0=ALU.mult, op1=ALU.add,
        )
        # one Newton step: seed error ~3.4% -> ~0.17%, well under the bf16
        # h-storage quantization; a second step would only stretch the
        # latency ladder that gates the first qkv matmul
        yw = small.tile([128, CT], F32, tag="yw", name="yw")
        nc.vector.tensor_mul(yw[:], y0[:], y0[:])
        nc.vector.tensor_mul(yw[:], yw[:], hv[:])
        nc.vector.tensor_scalar_add(yw[:], yw[:], 1.5)
        rstd = small.tile([128, CT], F32, tag="rstd", name="rstd")
        nc.vector.tensor_mul(rstd[:], y0[:], yw[:])
        # h = x*A + B with A = rstd*gamma, B = beta - gmean*A
        nc.vector.tensor_mul(A_s[s][:], rstd[:], gam[:])
        tmpA = small.tile([128, CT], F32, tag="tmpA", name="tmpA")
        nc.vector.tensor_mul(tmpA[:], gm[:], A_s[s][:])
        nc.vector.tensor_sub(B_s[s][:], bet[:], tmpA[:])

    # Main pools
    hpool = ctx.enter_context(tc.tile_pool(name="h", bufs=1))
    qkpool = ctx.enter_context(tc.tile_pool(name="qk", bufs=3))
    vtpool = ctx.enter_context(tc.tile_pool(name="vt", bufs=2))
    atpool = ctx.enter_context(tc.tile_pool(name="attn", bufs=2))
    aopool = ctx.enter_context(tc.tile_pool(name="ao", bufs=1))
    rbpool = ctx.enter_context(tc.tile_pool(name="rb", bufs=1))
    rcppool = ctx.enter_context(tc.tile_pool(name="rcps", bufs=1))
    outpool = ctx.enter_context(tc.tile_pool(name="outp", bufs=2))

    def emit_prep(s):
        # ---- groupnorm apply -> h (bf16) on DVE (ACT carries the copies) ----
        h_sb = hpool.tile([128, CT, N], BF16, tag="h", name="h_sb")
        for t in range(CT):
            nc.vector.tensor_scalar(
                out=h_sb[:, t, :], in0=x_sb[:, s * CT + t, :],
                scalar1=A_s[s][:, t : t + 1], scalar2=B_s[s][:, t : t + 1],
                op0=ALU.mult, op1=ALU.add,
            )

        # ---- qk = wqkT.T @ h   ([o, n], o-tile p holds heads 2p, 2p+1) ----
        q_sb = qkpool.tile([128, CT, N], BF16, tag="q", name="q_sb")
        k_sb = qkpool.tile([128, CT, N], BF16, tag="k", name="k_sb")
        for o in range(2 * CT):
            for nh in range(NHALF):
                ps = ps_acc.tile([128, 512], F32, tag="acc", name="ps")
                for kt in range(CT):
                    nc.tensor.matmul(
                        ps[:],
                        wqk_sb[:, kt, o * 128 : (o + 1) * 128],
                        h_sb[:, kt, nh * 512 : (nh + 1) * 512],
                        start=(kt == 0),
                        stop=(kt == CT - 1),
                    )
                if o < CT:  # q: ACT copy, scale+pre-scaled-bias fused (host
                    # sends bq * d^-0.5, so out = ps*s + bq*s = (ps+bq)*s)
                    nc.scalar.activation(
                        q_sb[:, o, nh * 512 : (nh + 1) * 512], ps[:],
                        AF.Identity, bias=bq_sb[:, o, :], scale=float(D) ** -0.5,
                    )
                else:  # k channels: plain ACT copy (bias dropped, see header)
                    nc.scalar.activation(
                        k_sb[:, o - CT, nh * 512 : (nh + 1) * 512], ps[:], AF.Copy
                    )

        # ---- vT = h.T @ wvT  ([m, dv] + ones column for colsum), f16 ----
        vt_sb = vtpool.tile([128, MT, HEADS, D + 1], F16, tag="vt")
        nc.vector.memset(vt_sb[:, :, :, D : D + 1], 1.0)
        for m in range(MT):
            ps = ps_acc.tile([128, 512], F32, tag="acc", name="ps")
            for kt in range(CT):
                nc.tensor.matmul(
                    ps[:],
                    h_sb[:, kt, m * 128 : (m + 1) * 128],
                    wv_sb[:, kt, :],
                    start=(kt == 0),
                    stop=(kt == CT - 1),
                )
            nc.scalar.activation(
                vt_sb[:, m, :, 0:D],
                ps[:].rearrange("p (h d) -> p h d", h=HEADS),
                AF.Copy,
            )

        return q_sb, k_sb, vt_sb

    def emit_attention(s, q_sb, k_sb, vt_sb, fast_drain):
        # ---- attention: QK/exp of pair p interleaved with AV of pair p-1 ----
        ao_sb = aopool.tile([128, CT, N], BF16, tag="ao", name="ao_sb")

        def emit_av_chunk(prev_state, m):
            p0, at0, avs0 = prev_state
            for hh in range(2):
                for nh in range(NHALF):
                    nc.tensor.matmul(
                        avs0[hh][nh][:],
                        vt_sb[:, m, 2 * p0 + hh, :],
                        at0[:, hh, m, nh * 512 : (nh + 1) * 512],
                        start=(m == 0),
                        stop=(m == MT - 1),
                    )

        def emit_normalize(prev_state):
            # custom-DVE recip misreads PSUM sources on HW: SBUF-bounce.
            # one recip per head (both n-halves gathered) halves the chain.
            p0, at0, avs0 = prev_state
            for hh in range(2):
                cs = rcppool.tile([1, 1024], F32, tag="cs", name="cs")
                for nh in range(NHALF):
                    nc.vector.tensor_copy(
                        cs[0:1, nh * 512 : (nh + 1) * 512],
                        avs0[hh][nh][D : D + 1, :],
                    )
                rcp = rcppool.tile([1, 1024], F32, tag="rcp", name="rcp")
                nc.vector.reciprocal_approx_fast(rcp[:], cs[:])
                for nh in range(NHALF):
                    nsl = slice(nh * 512, (nh + 1) * 512)
                    rb = rbpool.tile([64, 512], F32, tag="rb", name="rb")
                    nc.gpsimd.partition_broadcast(
                        rb[:], rcp[0:1, nh * 512 : (nh + 1) * 512]
                    )
                    nc.vector.tensor_mul(
                        ao_sb[hh * 64 : (hh + 1) * 64, p0, nsl],
                        avs0[hh][nh][0:D, :],
                        rb[:],
                    )

        prev = None
        for p in range(HEADS // 2):
            at_pair = atpool.tile([128, 2, MT, N], F16, tag="attn", name="at_pair")
            for m in range(MT):
                for hh in range(2):
                    base = hh * 64
                    sc = ps_sc.tile([128, N], F32, tag="sc", name="sc")
                    for nh in range(NHALF):
                        nsl = slice(nh * 512, (nh + 1) * 512)
                        nc.tensor.matmul(
                            sc[:, nsl],
                            k_sb[base : base + 64, p, m * 128 : (m + 1) * 128],
                            q_sb[base : base + 64, p, nsl],
                            start=True,
                            stop=True,
                            tile_position=(base, 0),
                        )
                    if hh in DVE_EXP_HH:
                        nc.vector.tensor_scalar(
                            out=at_pair[:, hh, m, :].bitcast(I16),
                            in0=sc[:],
                            scalar1=SCH_K,
                            scalar2=SCH_C,
                            op0=ALU.mult,
                            op1=ALU.add,
                        )
                    else:
                        nc.scalar.activation(at_pair[:, hh, m, :], sc[:], AF.Exp)
                if prev is not None:
                    emit_av_chunk(prev, m)
            if prev is not None:
                emit_normalize(prev)
            if p < HEADS // 2 - 1:
                avs = [
                    [
                        ps_acc.tile([D + 1, 512], F32, tag="acc", name=f"av{hh}_{nh}")
                        for nh in range(NHALF)
                    ]
                    for hh in range(2)
                ]
            else:
                # drain pair accumulates in the (then idle) scores pool so the
                # acc pool frees for the next sample's qkv before normalize
                dr = [
                    ps_sc.tile([D + 1, N], F32, tag="sc", name=f"drain{hh}")
                    for hh in range(2)
                ]
                avs = [
                    [dr[hh][:, nh * 512 : (nh + 1) * 512] for nh in range(NHALF)]
                    for hh in range(2)
                ]
            prev = (p, at_pair, avs)
        for m in range(MT):
            emit_av_chunk(prev, m)
        if fast_drain:
            return ao_sb, prev
        emit_normalize(prev)
        return ao_sb, None

    # output stores round-robin across engine DMA queues: a single queue
    # serializes the 16 x 256KB tiles (~1.3us each) and dominates the tail
    out_eng = [nc.gpsimd, nc.sync, nc.scalar]
    out_ctr = [0]

    def emit_out_dma(dst, src):
        out_eng[out_ctr[0] % 3].dma_start(out=dst, in_=src)
        out_ctr[0] += 1

    def emit_normalize_fast(ao_sb, drain_state, nh):
        # final-drain path: PE broadcast of both heads' reciprocals.
        # Both colsums land in one partition-0 row (partition-offset writes
        # must be 32-aligned); two accumulating 1-contraction f32 matmuls
        # splat them across the 64-row blocks of rb_ps.
        p0, at0, avs0 = drain_state
        nsl = slice(nh * 512, (nh + 1) * 512)
        cs2 = rcppool.tile([1, 1024], F32, tag="cs", name="cs2")
        for hh in range(2):
            nc.vector.tensor_copy(
                cs2[0:1, hh * 512 : (hh + 1) * 512], avs0[hh][nh][D : D + 1, :]
            )
        rcp2 = rcppool.tile([1, 1024], F32, tag="rcp", name="rcp2")
        nc.vector.reciprocal_approx_fast(rcp2[:], cs2[:])
        rb_ps = ps_acc.tile([128, 512], F32, tag="acc", name="rb_ps")
        nc.tensor.matmul(rb_ps[:], sel_lo[:], rcp2[0:1, 0:512], start=True, stop=False)
        nc.tensor.matmul(rb_ps[:], sel_hi[:], rcp2[0:1, 512:1024], start=False, stop=True)
        rb = rbpool.tile([128, 512], F32, tag="rb2", name="rb2")
        nc.vector.tensor_copy(rb[:], rb_ps[:])
        for hh in range(2):
            nc.vector.tensor_mul(
                ao_sb[hh * 64 : (hh + 1) * 64, p0, nsl],
                avs0[hh][nh][0:D, :],
                rb[hh * 64 : (hh + 1) * 64, :],
            )

    def emit_proj(s, ao_sb):
        # ---- proj + bias + residual, two waves of 4 open psum groups ----
        # kt=0..2 partials need only pairs 0-2's ao, so they run while the
        # drain pair's normalize chain resolves; kt=3 closes each group.
        for wave in range(2):
            pss = []
            for t in range(wave * 2, wave * 2 + 2):
                for nh in range(NHALF):
                    nsl = slice(nh * 512, (nh + 1) * 512)
                    ps = ps_acc.tile([128, 512], F32, tag="acc", name=f"pj{t}_{nh}")
                    pss.append((t, nh, nsl, ps))
                    for kt in range(CT - 1):
                        nc.tensor.matmul(
                            ps[:],
                            wp_sb[:, kt, t * 128 : (t + 1) * 128],
                            ao_sb[:, kt, nsl],
                            start=(kt == 0),
                            stop=False,
                        )
            for t, nh, nsl, ps in pss:
                nc.tensor.matmul(
                    ps[:],
                    wp_sb[:, CT - 1, t * 128 : (t + 1) * 128],
                    ao_sb[:, CT - 1, nsl],
                    start=False,
                    stop=True,
                )
                ot = outpool.tile([128, 512], F32, tag="out", name="ot")
                nc.vector.scalar_tensor_tensor(
                    out=ot[:],
                    in0=ps[:],
                    scalar=beff_sb[:, t, :],
                    in1=x_sb[:, s * CT + t, nsl],
                    op0=ALU.add,
                    op1=ALU.add,
                )
                emit_out_dma(out_e[s, t * 128 : (t + 1) * 128, nsl], ot[:])

    def emit_proj_final(s, ao_sb, drain_state):
        # ---- final sample: per-nh drain normalize + proj + store ----
        # kt=0..2 partials only need pairs 0-2's ao, so they run while the
        # drain normalize resolves; each nh's kt=3 closes after its own.
        emit_normalize_fast(ao_sb, drain_state, 0)
        for nh in range(NHALF):
            nsl = slice(nh * 512, (nh + 1) * 512)
            pss = []
            for t in range(CT):
                ps = ps_acc.tile([128, 512], F32, tag="acc", name=f"pjf{t}_{nh}")
                pss.append((t, ps))
                for kt in range(CT - 1):
                    nc.tensor.matmul(
                        ps[:],
                        wp_sb[:, kt, t * 128 : (t + 1) * 128],
                        ao_sb[:, kt, nsl],
                        start=(kt == 0),
                        stop=False,
                    )
            if nh == 0:
                emit_normalize_fast(ao_sb, drain_state, 1)
            for t, ps in pss:
                nc.tensor.matmul(
                    ps[:],
                    wp_sb[:, CT - 1, t * 128 : (t + 1) * 128],
                    ao_sb[:, CT - 1, nsl],
                    start=False,
                    stop=True,
                )
                ot = outpool.tile([128, 512], F32, tag="out", name="ot")
                nc.vector.scalar_tensor_tensor(
                    out=ot[:],
                    in0=ps[:],
                    scalar=beff_sb[:, t, :],
                    in1=x_sb[:, s * CT + t, nsl],
                    op0=ALU.add,
                    op1=ALU.add,
                )
                emit_out_dma(out_e[s, t * 128 : (t + 1) * 128, nsl], ot[:])

    # Drive: emit next sample's qkv prep between a sample's attention drain
    # and its proj, so the PE instruction stream has work while the
    # normalize latency resolves. Both samples' stats run in the prologue
    # (the group-aggregation psum is free there; it is not during attention).
    emit_stats(0)
    tiles = emit_prep(0)
    for s in range(BS):
        final = s == BS - 1
        ao, drain_state = emit_attention(s, *tiles, final)
        if not final:
            # sample-1 stats only now (see DMA section comment): by this
            # point their x DMAs are long done, so the strict-FIFO DVE queue
            # can never head-of-line-block on them.
            for t in range(CT):
                j = (s + 1) * CT + t
                for hc in range(2):
                    nc.vector.bn_stats(
                        out=stats_sb[:, j, hc, :],
                        in_=x_sb[:, j, hc * 512 : (hc + 1) * 512],
                    )
            emit_stats(s + 1)
            tiles = emit_prep(s + 1)
            emit_proj(s, ao)
        else:
            emit_proj_final(s, ao, drain_state)


def build_bass() -> bass.Bass:
    nc = bacc.Bacc()
    te = {
        "x": nc.declare_dram_parameter("x", [BS, C, N], BF16, isOutput=False),
        "wqkT": nc.declare_dram_parameter("wqkT", [C, 2 * C], BF16, isOutput=False),
        "wvT": nc.declare_dram_parameter("wvT", [C, C], BF16, isOutput=False),
        "wpT": nc.declare_dram_parameter("wpT", [C, C], BF16, isOutput=False),
        "bq": nc.declare_dram_parameter("bq", [C, 1], F32, isOutput=False),
        "beff": nc.declare_dram_parameter("beff", [C, 1], F32, isOutput=False),
        "gamma": nc.declare_dram_parameter("gamma", [C, 1], F32, isOutput=False),
        "beta": nc.declare_dram_parameter("beta", [C, 1], F32, isOutput=False),
        "selg": nc.declare_dram_parameter("selg", [128, 128], F32R, isOutput=False),
        "out": nc.declare_dram_parameter("out", [BS, C, N], F32, isOutput=True),
    }
    with tile.TileContext(nc) as tc:
        with ExitStack() as ctx:
            _build_tile(ctx, tc, te)
    nc.finalize()
    return nc


def _make_selg() -> np.ndarray:
    # block-diagonal group-average matrix: selg[i, j] = 1/16 if same 16-ch group
    selg = np.zeros((128, 128), np.float32)
    for j in range(128):
        g0 = (j // 16) * 16
        selg[g0 : g0 + 16, j] = 1.0 / 16.0
    return selg


def make_in_maps(inputs: dict) -> list[dict]:
    x = np.ascontiguousarray(np.asarray(inputs["x"], np.float32)).reshape(B, C, N)
    w_qkv = np.asarray(inputs["w_qkv"], np.float32)
    b_qkv = np.asarray(inputs["b_qkv"], np.float32)
    w_proj = np.asarray(inputs["w_proj"], np.float32)
    b_proj = np.asarray(inputs["b_proj"], np.float32)
    gamma = np.asarray(inputs["gamma"], np.float32)
    beta = np.asarray(inputs["beta"], np.float32)

    bf = ml_dtypes.bfloat16
    common = {
        "wqkT": np.ascontiguousarray(w_qkv[: 2 * C, :].T).astype(bf),
        "wvT": np.ascontiguousarray(w_qkv[2 * C :, :].T).astype(bf),
        "wpT": np.ascontiguousarray(w_proj.T).astype(bf),
        "bq": (b_qkv[:C] * float(D) ** -0.5).reshape(C, 1).astype(np.float32),
        "beff": (b_proj + w_proj @ b_qkv[2 * C :]).reshape(C, 1).astype(np.float32),
        "gamma": gamma.reshape(C, 1).copy(),
        "beta": beta.reshape(C, 1).copy(),
        "selg": _make_selg(),
    }
    x_bf = x.astype(bf)
    return [
        {"x": np.ascontiguousarray(x_bf[i * BS : (i + 1) * BS]), **common}
        for i in range(NCORES)
    ]


def kernel(**inputs) -> np.ndarray:
    global LAST_EXEC_NS, LAST_RESULTS
    nc = build_bass()
    in_maps = make_in_maps(inputs)
    res = run_bass_kernel_spmd(nc, in_maps, list(range(NCORES)))
    LAST_RESULTS = res
    LAST_EXEC_NS = res.exec_time_ns
    out = np.concatenate([np.asarray(res.results[i]["out"]) for i in range(NCORES)], 0)
    return out.reshape(B, C, H, W).astype(np.float32)


# revision 51
# speedup vs baseline: 1.2793x; 1.2793x over previous
"""Trainium2 Bass kernel: AttentionBlock (GroupNorm + 1x1-conv QKV + MHA + proj + residual).

Data-parallel over batch: 16 samples -> 8 NeuronCores x 2 samples. Each core
runs the whole block locally (attention is per-sample, no collectives); the
host shards inputs and concatenates the 8 output shards.

Math notes (exact rewrites, not approximations):
  - scores are computed transposed, S^T[m,n] = sum_d k[d,m] q'[d,n] with
    q' = (q + b_q) * d^-0.5. The k-bias adds a column-constant to S^T which
    softmax cancels, so it is dropped.
  - softmax denominator comes from a ones-column appended to v^T in the
    attn@v matmul (row 64 of the [65, n] output accumulates colsum(exp S^T)).
  - v-bias: attn rows sum to 1, so  attn @ (Wv h + bv) = attn @ Wv h + bv;
    the bv term is folded into the proj bias on the host:
    beff = b_proj + w_proj @ bv.
  - groupnorm stats come from per-channel bn_stats on the resident x tiles;
    16-channel group aggregation is a tiny block-diagonal (1/16) matmul that
    simultaneously broadcasts the group stats back to channel lanes.

Approximation (bounded, validated vs the 2e-2 gate at ~7e-4 end-to-end):
  - half the softmax exps (hh=1 of each head pair) use a one-op Schraudolph
    exponential on DVE: f16 bits = round(logit * 1024/ln2 + 15300.7), i.e.
    2^(x/ln2) with a linearly-filled mantissa (max rel err ~3%, mean ~0).
    Softmax renormalization cancels the mean error; the weighted average over
    ~1e2-effective attention positions suppresses the remainder ~30x.

Performance structure (HAM: PE clock is 1.2 GHz until ~3.4us of continuous
activity, 2.4 GHz after; idle windows re-throttle):
  - a warmup chain of dummy matmuls keeps the PE busy from t~0 so the real
    stream starts (and stays) at 2.4 GHz,
  - per-sample stats gating: sample 0's qkv starts as soon as its 2MB of x
    has streamed in (~9us), not after the full batch stats,
  - exp split ACT/DVE makes attention PE-bound instead of ACT-bound,
  - the final drain normalize uses a PE broadcast (f32 selector matmul) of
    the softmax reciprocals instead of the 1us GpSimd partition_broadcast.
"""

import os
import sys
import types
from contextlib import ExitStack

import ml_dtypes
import numpy as np

# If BASS_TRACE is set but this container's antenv lacks the NTFF hook
# module, bass_utils' trace path would crash on import; give it a null
# hook so tracing degrades gracefully instead.
try:
    import antenv.axon_hooks  # noqa: F401
except Exception:  # pragma: no cover
    try:
        import antenv

        _hookmod = types.ModuleType("antenv.axon_hooks")
        _hook = [None]
        _hookmod.set_axon_ntff_profile_hook = lambda h: _hook.__setitem__(0, h)
        _hookmod.get_axon_ntff_profile_hook = lambda: _hook[0]
        sys.modules["antenv.axon_hooks"] = _hookmod
        antenv.axon_hooks = _hookmod
    except Exception:
        pass

import concourse.bass as bass
import concourse.tile as tile
from concourse import bacc
from concourse import mybir
from concourse.bass_utils import run_bass_kernel_spmd

F32 = mybir.dt.float32
F32R = mybir.dt.float32r
BF16 = mybir.dt.bfloat16
F16 = mybir.dt.float16
I16 = mybir.dt.int16
I32 = mybir.dt.int32
AF = mybir.ActivationFunctionType
ALU = mybir.AluOpType

# Problem dims (hardcoded per spec: x [16, 512, 32, 32] f32)
B, C, H, W = 16, 512, 32, 32
N = H * W                # 1024 spatial positions
NCORES = 8
BS = B // NCORES         # 2 samples per core
G = 32                   # groupnorm groups
HEADS = 8
D = C // HEADS           # 64
CT = C // 128            # 4 channel tiles
MT = N // 128            # 8 m-tiles (spatial, attention contraction)
NHALF = 2                # n split in halves of 512 (psum bank limit)
EPS = 1e-5

# Schraudolph f16-bits exp: bits = round(x * 1024/ln2 + (15360 - 59.3))
SCH_K = 1024.0 / float(np.log(2.0))
SCH_C = 15360.0 - 59.3
# exp engine per (pair-half): hh0 exact on ACT, hh1 Schraudolph on DVE
DVE_EXP_HH = {1}
N_WARMUP = 75            # dummy matmuls covering the prologue (~9.5us)

LAST_EXEC_NS = None
LAST_RESULTS = None


def _build_tile(ctx: ExitStack, tc: tile.TileContext, te: dict):
    nc = tc.nc
    x_e, out_e = te["x"], te["out"]

    const = ctx.enter_context(tc.tile_pool(name="const", bufs=1))
    small = ctx.enter_context(tc.tile_pool(name="small", bufs=8))

    # ---- PE warmup: serial dummy matmul chain from t~0 keeps HAM at 8/8 ----
    # (own psum pool, closed before the main psum pools open: 8-bank budget)
    warm_w = const.tile([128, 256], BF16)
    nc.vector.memset(warm_w[:], 0.0)
    with tc.tile_pool(name="warm", bufs=1, space="PSUM") as warmp:
        warm_ps = warmp.tile([128, 256], F32, name="warm_ps")
        for _ in range(N_WARMUP):
            nc.tensor.matmul(
                warm_ps[:], warm_w[:, 0:128], warm_w[:], start=True, stop=True
            )
    ps_acc = ctx.enter_context(tc.tile_pool(name="ps_acc", bufs=4, space="PSUM"))
    ps_sc = ctx.enter_context(tc.tile_pool(name="ps_sc", bufs=2, space="PSUM"))

    # preload the Exp ACT table set off the critical path
    eps_sb = const.tile([1, 1], F32)
    nc.vector.memset(eps_sb[:], EPS)
    dummy_act = const.tile([1, 1], F32)
    nc.scalar.activation(dummy_act[:], eps_sb[:], AF.Exp)

    # ---- x into SBUF (single pass) + weights, spread over 4 DMA queues ----
    # Queue plan (per engine queue, in issue order): sample-0 x tile, then a
    # wqk k-tile, small constants, sample-1 x tile, then wv/wp. Sample 0's
    # stats + first qkv matmuls gate on ~770KB per queue (~9us), not on the
    # whole 6MB.
    xpool = ctx.enter_context(tc.tile_pool(name="xres", bufs=1))
    x_sb = xpool.tile([128, BS * CT, N], BF16)
    stats_sb = const.tile([128, BS * CT, 2, 6], F32)
    wqk_sb = const.tile([128, CT, 2 * C], BF16)   # w_qkv[:1024].T tiles
    wv_sb = const.tile([128, CT, C], BF16)        # w_qkv[1024:].T tiles
    wp_sb = const.tile([128, CT, C], BF16)        # w_proj.T tiles
    bq_sb = const.tile([128, CT, 1], F32)
    beff_sb = const.tile([128, CT, 1], F32)
    gam = const.tile([128, CT], F32)
    bet = const.tile([128, CT], F32)
    selg_sb = const.tile([128, 128], F32R)        # block-diag 1/16 group avg
    dma_eng = [nc.sync, nc.gpsimd, nc.scalar, nc.sync]

    # sample-0 x in 512-col half-chunks, round-robin over the 3 DMA-capable
    # queues, bn_stats right behind each chunk: the stats chain tracks the
    # stream instead of gating on the slowest whole tile.
    qi = 0
    for t in range(CT):
        for hc in range(2):
            dma_eng[qi % 3].dma_start(
                out=x_sb[:, t, hc * 512 : (hc + 1) * 512],
                in_=x_e[0, t * 128 : (t + 1) * 128, hc * 512 : (hc + 1) * 512],
            )
            qi += 1
            nc.vector.bn_stats(
                out=stats_sb[:, t, hc, :],
                in_=x_sb[:, t, hc * 512 : (hc + 1) * 512],
            )
    for kt in range(CT):
        sl = slice(kt * 128, (kt + 1) * 128)
        dma_eng[kt].dma_start(out=wqk_sb[:, kt, :], in_=te["wqkT"][sl, :])
    nc.sync.dma_start(out=selg_sb[:], in_=te["selg"][:])
    nc.sync.dma_start(
        out=gam[:], in_=bass.AP(tensor=te["gamma"], offset=0, ap=[[1, 128], [128, CT]])
    )
    nc.sync.dma_start(
        out=bet[:], in_=bass.AP(tensor=te["beta"], offset=0, ap=[[1, 128], [128, CT]])
    )
    for kt in range(CT):
        sl = slice(kt * 128, (kt + 1) * 128)
        nc.gpsimd.dma_start(out=bq_sb[:, kt, :], in_=te["bq"][sl, :])
    # sample-1 x DMAs issue now (queue order), but their bn_stats are
    # emitted AFTER emit_prep(0): DVE is strict FIFO, and a bn_stats
    # waiting on a late DMA would head-of-line-block sample 0's stat chain.
    for t in range(CT):
        j = CT + t
        dma_eng[t].dma_start(
            out=x_sb[:, j, :], in_=x_e[1, t * 128 : (t + 1) * 128, :]
        )
    for kt in range(CT):
        sl = slice(kt * 128, (kt + 1) * 128)
        dma_eng[kt].dma_start(out=wv_sb[:, kt, :], in_=te["wvT"][sl, :])
    for kt in range(CT):
        sl = slice(kt * 128, (kt + 1) * 128)
        dma_eng[kt].dma_start(out=wp_sb[:, kt, :], in_=te["wpT"][sl, :])
        nc.scalar.dma_start(out=beff_sb[:, kt, :], in_=te["beff"][sl, :])

    # selectors for the final-drain reciprocal broadcast (partition-0 rows;
    # partition-offset writes must be 32-aligned, so two separate tiles)
    sel_lo = const.tile([1, 128], F32)
    nc.vector.memset(sel_lo[:], 0.0)
    nc.vector.memset(sel_lo[0:1, 0:64], 1.0)
    sel_hi = const.tile([1, 128], F32)
    nc.vector.memset(sel_hi[:], 0.0)
    nc.vector.memset(sel_hi[0:1, 64:128], 1.0)

    # ---- per-sample groupnorm stat chain -> A (scale), B (shift) ----
    A_s = [const.tile([128, CT], F32, name=f"A_{s}") for s in range(BS)]
    B_s = [const.tile([128, CT], F32, name=f"B_{s}") for s in range(BS)]

    def emit_stats(s):
        mv = small.tile([128, CT, 2], F32, tag="mv", name="mv")
        for t in range(CT):
            nc.vector.bn_aggr(out=mv[:, t, :], in_=stats_sb[:, s * CT + t, :, :])
        # per-channel (mean, E[x^2]) in f32r for the group-average matmul
        ex2 = small.tile([128, CT], F32, tag="ex2", name="ex2")
        nc.vector.tensor_mul(ex2[:], mv[:, :, 0], mv[:, :, 0])
        nc.vector.tensor_add(ex2[:], ex2[:], mv[:, :, 1])
        pcs = small.tile([128, CT, 2], F32R, tag="pcs", name="pcs")
        nc.vector.tensor_copy(pcs[:, :, 0], mv[:, :, 0])
        nc.vector.tensor_copy(pcs[:, :, 1], ex2[:])
        gst = ps_sc.tile([128, CT, 2], F32, tag="sc", name="gst")
        for t in range(CT):
            nc.tensor.matmul(
                gst[:, t, :], selg_sb[:], pcs[:, t, :], start=True, stop=True
            )
        # gvar = gEx2 - gmean^2 ; rstd via int-seed + 2 Newton steps (DVE only)
        # (gmean bounced to SBUF: DVE has a single PSUM read port)
        gm = small.tile([128, CT], F32, tag="gm", name="gm")
        nc.vector.tensor_copy(gm[:], gst[:, :, 0])
        vpe = small.tile([128, CT], F32, tag="vpe", name="vpe")
        nc.vector.tensor_mul(vpe[:], gm[:], gm[:])
        nc.vector.tensor_sub(vpe[:], gst[:, :, 1], vpe[:])
        nc.vector.tensor_scalar_add(vpe[:], vpe[:], EPS)
        hv = small.tile([128, CT], F32, tag="hv", name="hv")
        nc.vector.tensor_scalar_mul(hv[:], vpe[:], -0.5)
        y0 = small.tile([128, CT], F32, tag="y0", name="y0")
        ysh = small.tile([128, CT], I32, tag="ysh", name="ysh")
        nc.vector.tensor_scalar(
            out=ysh[:], in0=vpe[:].bitcast(I32), scalar1=1, scalar2=None,
            op0=ALU.arith_shift_right,
        )
        nc.vector.tensor_scalar(
            out=y0[:].bitcast(I32), in0=ysh[:], scalar1=-1, scalar2=0x5F3759DF,
            op0=ALU.mult, op1=ALU.add,
        )
        # one Newton step: seed error ~3.4% -> ~0.17%, well under the bf16
        # h-storage quantization; a second step would only stretch the
        # latency ladder that gates the first qkv matmul
        yw = small.tile([128, CT], F32, tag="yw", name="yw")
        nc.vector.tensor_mul(yw[:], y0[:], y0[:])
        nc.vector.tensor_mul(yw[:], yw[:], hv[:])
        nc.vector.tensor_scalar_add(yw[:], yw[:], 1.5)
        rstd = small.tile([128, CT], F32, tag="rstd", name="rstd")
        nc.vector.tensor_mul(rstd[:], y0[:], yw[:])
        # h = x*A + B with A = rstd*gamma, B = beta - gmean*A
        nc.vector.tensor_mul(A_s[s][:], rstd[:], gam[:])
        tmpA = small.tile([128, CT], F32, tag="tmpA", name="tmpA")
        nc.vector.tensor_mul(tmpA[:], gm[:], A_s[s][:])
        nc.vector.tensor_sub(B_s[s][:], bet[:], tmpA[:])

    # Main pools
    hpool = ctx.enter_context(tc.tile_pool(name="h", bufs=1))
    qkpool = ctx.enter_context(tc.tile_pool(name="qk", bufs=3))
    vtpool = ctx.enter_context(tc.tile_pool(name="vt", bufs=2))
    atpool = ctx.enter_context(tc.tile_pool(name="attn", bufs=2))
    aopool = ctx.enter_context(tc.tile_pool(name="ao", bufs=1))
    rbpool = ctx.enter_context(tc.tile_pool(name="rb", bufs=1))
    rcppool = ctx.enter_context(tc.tile_pool(name="rcps", bufs=1))
    outpool = ctx.enter_context(tc.tile_pool(name="outp", bufs=2))

    def emit_prep(s):
        # ---- groupnorm apply -> h (bf16) on DVE (ACT carries the copies) ----
        h_sb = hpool.tile([128, CT, N], BF16, tag="h", name="h_sb")
        for t in range(CT):
            nc.vector.tensor_scalar(
                out=h_sb[:, t, :], in0=x_sb[:, s * CT + t, :],
                scalar1=A_s[s][:, t : t + 1], scalar2=B_s[s][:, t : t + 1],
                op0=ALU.mult, op1=ALU.add,
            )

        # ---- qk = wqkT.T @ h   ([o, n], o-tile p holds heads 2p, 2p+1) ----
        q_sb = qkpool.tile([128, CT, N], BF16, tag="q", name="q_sb")
        k_sb = qkpool.tile([128, CT, N], BF16, tag="k", name="k_sb")
        for o in range(2 * CT):
            for nh in range(NHALF):
                ps = ps_acc.tile([128, 512], F32, tag="acc", name="ps")
                for kt in range(CT):
                    nc.tensor.matmul(
                        ps[:],
                        wqk_sb[:, kt, o * 128 : (o + 1) * 128],
                        h_sb[:, kt, nh * 512 : (nh + 1) * 512],
                        start=(kt == 0),
                        stop=(kt == CT - 1),
                    )
                if o < CT:  # q: ACT copy, scale+pre-scaled-bias fused (host
                    # sends bq * d^-0.5, so out = ps*s + bq*s = (ps+bq)*s)
                    nc.scalar.activation(
                        q_sb[:, o, nh * 512 : (nh + 1) * 512], ps[:],
                        AF.Identity, bias=bq_sb[:, o, :], scale=float(D) ** -0.5,
                    )
                else:  # k channels: plain ACT copy (bias dropped, see header)
                    nc.scalar.activation(
                        k_sb[:, o - CT, nh * 512 : (nh + 1) * 512], ps[:], AF.Copy
                    )

        # ---- vT = h.T @ wvT  ([m, dv] + ones column for colsum), f16 ----
        vt_sb = vtpool.tile([128, MT, HEADS, D + 1], F16, tag="vt")
        nc.vector.memset(vt_sb[:, :, :, D : D + 1], 1.0)
        for m in range(MT):
            ps = ps_acc.tile([128, 512], F32, tag="acc", name="ps")
            for kt in range(CT):
                nc.tensor.matmul(
                    ps[:],
                    h_sb[:, kt, m * 128 : (m + 1) * 128],
                    wv_sb[:, kt, :],
                    start=(kt == 0),
                    stop=(kt == CT - 1),
                )
            nc.scalar.activation(
                vt_sb[:, m, :, 0:D],
                ps[:].rearrange("p (h d) -> p h d", h=HEADS),
                AF.Copy,
            )

        return q_sb, k_sb, vt_sb

    def emit_attention(s, q_sb, k_sb, vt_sb, fast_drain):
        # ---- attention: QK/exp of pair p interleaved with AV of pair p-1 ----
        ao_sb = aopool.tile([128, CT, N], BF16, tag="ao", name="ao_sb")

        def emit_av_chunk(prev_state, m):
            p0, at0, avs0 = prev_state
            for hh in range(2):
                for nh in range(NHALF):
                    nc.tensor.matmul(
                        avs0[hh][nh][:],
                        vt_sb[:, m, 2 * p0 + hh, :],
                        at0[:, hh, m, nh * 512 : (nh + 1) * 512],
                        start=(m == 0),
                        stop=(m == MT - 1),
                    )

        def emit_normalize(prev_state):
            # custom-DVE recip misreads PSUM sources on HW: SBUF-bounce.
            # one recip per head (both n-halves gathered) halves the chain.
            p0, at0, avs0 = prev_state
            for hh in range(2):
                cs = rcppool.tile([1, 1024], F32, tag="cs", name="cs")
                for nh in range(NHALF):
                    nc.vector.tensor_copy(
                        cs[0:1, nh * 512 : (nh + 1) * 512],
                        avs0[hh][nh][D : D + 1, :],
                    )
                rcp = rcppool.tile([1, 1024], F32, tag="rcp", name="rcp")
                nc.vector.reciprocal_approx_fast(rcp[:], cs[:])
                for nh in range(NHALF):
                    nsl = slice(nh * 512, (nh + 1) * 512)
                    rb = rbpool.tile([64, 512], F32, tag="rb", name="rb")
                    nc.gpsimd.partition_broadcast(
                        rb[:], rcp[0:1, nh * 512 : (nh + 1) * 512]
                    )
                    nc.vector.tensor_mul(
                        ao_sb[hh * 64 : (hh + 1) * 64, p0, nsl],
                        avs0[hh][nh][0:D, :],
                        rb[:],
                    )

        prev = None
        for p in range(HEADS // 2):
            at_pair = atpool.tile([128, 2, MT, N], F16, tag="attn", name="at_pair")
            for m in range(MT):
                for hh in range(2):
                    base = hh * 64
                    sc = ps_sc.tile([128, N], F32, tag="sc", name="sc")
                    for nh in range(NHALF):
                        nsl = slice(nh * 512, (nh + 1) * 512)
                        nc.tensor.matmul(
                            sc[:, nsl],
                            k_sb[base : base + 64, p, m * 128 : (m + 1) * 128],
                            q_sb[base : base + 64, p, nsl],
                            start=True,
                            stop=True,
                            tile_position=(base, 0),
                        )
                    if hh in DVE_EXP_HH:
                        nc.vector.tensor_scalar(
                            out=at_pair[:, hh, m, :].bitcast(I16),
                            in0=sc[:],
                            scalar1=SCH_K,
                            scalar2=SCH_C,
                            op0=ALU.mult,
                            op1=ALU.add,
                        )
                    else:
                        nc.scalar.activation(at_pair[:, hh, m, :], sc[:], AF.Exp)
                if prev is not None:
                    emit_av_chunk(prev, m)
            if prev is not None:
                emit_normalize(prev)
            if p < HEADS // 2 - 1:
                avs = [
                    [
                        ps_acc.tile([D + 1, 512], F32, tag="acc", name=f"av{hh}_{nh}")
                        for nh in range(NHALF)
                    ]
                    for hh in range(2)
                ]
            else:
                # drain pair accumulates in the (then idle) scores pool so the
                # acc pool frees for the next sample's qkv before normalize
                dr = [
                    ps_sc.tile([D + 1, N], F32, tag="sc", name=f"drain{hh}")
                    for hh in range(2)
                ]
                avs = [
                    [dr[hh][:, nh * 512 : (nh + 1) * 512] for nh in range(NHALF)]
                    for hh in range(2)
                ]
            prev = (p, at_pair, avs)
        for m in range(MT):
            emit_av_chunk(prev, m)
        if fast_drain:
            return ao_sb, prev
        emit_normalize(prev)
        return ao_sb, None

    # output stores round-robin across engine DMA queues: a single queue
    # serializes the 16 x 256KB tiles (~1.3us each) and dominates the tail
    out_eng = [nc.gpsimd, nc.sync, nc.scalar]
    out_ctr = [0]

    def emit_out_dma(dst, src):
        out_eng[out_ctr[0] % 3].dma_start(out=dst, in_=src)
        out_ctr[0] += 1

    def emit_normalize_fast(ao_sb, drain_state, nh):
        # final-drain path: PE broadcast of both heads' reciprocals.
        # Both colsums land in one partition-0 row (partition-offset writes
        # must be 32-aligned); two accumulating 1-contraction f32 matmuls
        # splat them across the 64-row blocks of rb_ps.
        p0, at0, avs0 = drain_state
        nsl = slice(nh * 512, (nh + 1) * 512)
        cs2 = rcppool.tile([1, 1024], F32, tag="cs", name="cs2")
        for hh in range(2):
            nc.vector.tensor_copy(
                cs2[0:1, hh * 512 : (hh + 1) * 512], avs0[hh][nh][D : D + 1, :]
            )
        rcp2 = rcppool.tile([1, 1024], F32, tag="rcp", name="rcp2")
        nc.vector.reciprocal_approx_fast(rcp2[:], cs2[:])
        rb_ps = ps_acc.tile([128, 512], F32, tag="acc", name="rb_ps")
        nc.tensor.matmul(rb_ps[:], sel_lo[:], rcp2[0:1, 0:512], start=True, stop=False)
        nc.tensor.matmul(rb_ps[:], sel_hi[:], rcp2[0:1, 512:1024], start=False, stop=True)
        rb = rbpool.tile([128, 512], F32, tag="rb2", name="rb2")
        nc.vector.tensor_copy(rb[:], rb_ps[:])
        for hh in range(2):
            nc.vector.tensor_mul(
                ao_sb[hh * 64 : (hh + 1) * 64, p0, nsl],
                avs0[hh][nh][0:D, :],
                rb[hh * 64 : (hh + 1) * 64, :],
            )

    def emit_proj(s, ao_sb):
        # ---- proj + bias + residual, two waves of 4 open psum groups ----
        # kt=0..2 partials need only pairs 0-2's ao, so they run while the
        # drain pair's normalize chain resolves; kt=3 closes each group.
        for wave in range(2):
            pss = []
            for t in range(wave * 2, wave * 2 + 2):
                for nh in range(NHALF):
                    nsl = slice(nh * 512, (nh + 1) * 512)
                    ps = ps_acc.tile([128, 512], F32, tag="acc", name=f"pj{t}_{nh}")
                    pss.append((t, nh, nsl, ps))
                    for kt in range(CT - 1):
                        nc.tensor.matmul(
                            ps[:],
                            wp_sb[:, kt, t * 128 : (t + 1) * 128],
                            ao_sb[:, kt, nsl],
                            start=(kt == 0),
                            stop=False,
                        )
            for t, nh, nsl, ps in pss:
                nc.tensor.matmul(
                    ps[:],
                    wp_sb[:, CT - 1, t * 128 : (t + 1) * 128],
                    ao_sb[:, CT - 1, nsl],
                    start=False,
                    stop=True,
                )
                ot = outpool.tile([128, 512], F32, tag="out", name="ot")
                nc.vector.scalar_tensor_tensor(
                    out=ot[:],
                    in0=ps[:],
                    scalar=beff_sb[:, t, :],
                    in1=x_sb[:, s * CT + t, nsl],
                    op0=ALU.add,
                    op1=ALU.add,
                )
                emit_out_dma(out_e[s, t * 128 : (t + 1) * 128, nsl], ot[:])

    def emit_proj_final(s, ao_sb, drain_state):
        # ---- final sample: per-nh drain normalize + proj + store ----
        # kt=0..2 partials only need pairs 0-2's ao, so they run while the
        # drain normalize resolves; each nh's kt=3 closes after its own.
        emit_normalize_fast(ao_sb, drain_state, 0)
        for nh in range(NHALF):
            nsl = slice(nh * 512, (nh + 1) * 512)
            pss = []
            for t in range(CT):
                ps = ps_acc.tile([128, 512], F32, tag="acc", name=f"pjf{t}_{nh}")
                pss.append((t, ps))
                for kt in range(CT - 1):
                    nc.tensor.matmul(
                        ps[:],
                        wp_sb[:, kt, t * 128 : (t + 1) * 128],
                        ao_sb[:, kt, nsl],
                        start=(kt == 0),
                        stop=False,
                    )
            if nh == 0:
                emit_normalize_fast(ao_sb, drain_state, 1)
            for t, ps in pss:
                nc.tensor.matmul(
                    ps[:],
                    wp_sb[:, CT - 1, t * 128 : (t + 1) * 128],
                    ao_sb[:, CT - 1, nsl],
                    start=False,
                    stop=True,
                )
                ot = outpool.tile([128, 512], F32, tag="out", name="ot")
                nc.vector.scalar_tensor_tensor(
                    out=ot[:],
                    in0=ps[:],
                    scalar=beff_sb[:, t, :],
                    in1=x_sb[:, s * CT + t, nsl],
                    op0=ALU.add,
                    op1=ALU.add,
                )
                emit_out_dma(out_e[s, t * 128 : (t + 1) * 128, nsl], ot[:])

    # Drive: emit next sample's qkv prep between a sample's attention drain
    # and its proj, so the PE instruction stream has work while the
    # normalize latency resolves. Both samples' stats run in the prologue
    # (the group-aggregation psum is free there; it is not during attention).
    emit_stats(0)
    tiles = emit_prep(0)
    # sample-1 stats here (not earlier: a bn_stats whose x DMA is still in
    # flight would head-of-line-block sample 0's chain in the strict-FIFO
    # DVE queue; not later: prep(1) gates on this chain's A/B)
    for t in range(CT):
        j = CT + t
        for hc in range(2):
            nc.vector.bn_stats(
                out=stats_sb[:, j, hc, :],
                in_=x_sb[:, j, hc * 512 : (hc + 1) * 512],
            )
    emit_stats(1)
    for s in range(BS):
        final = s == BS - 1
        ao, drain_state = emit_attention(s, *tiles, final)
        if not final:
            tiles = emit_prep(s + 1)
            emit_proj(s, ao)
        else:
            emit_proj_final(s, ao, drain_state)


def build_bass() -> bass.Bass:
    nc = bacc.Bacc()
    te = {
        "x": nc.declare_dram_parameter("x", [BS, C, N], BF16, isOutput=False),
        "wqkT": nc.declare_dram_parameter("wqkT", [C, 2 * C], BF16, isOutput=False),
        "wvT": nc.declare_dram_parameter("wvT", [C, C], BF16, isOutput=False),
        "wpT": nc.declare_dram_parameter("wpT", [C, C], BF16, isOutput=False),
        "bq": nc.declare_dram_parameter("bq", [C, 1], F32, isOutput=False),
        "beff": nc.declare_dram_parameter("beff", [C, 1], F32, isOutput=False),
        "gamma": nc.declare_dram_parameter("gamma", [C, 1], F32, isOutput=False),
        "beta": nc.declare_dram_parameter("beta", [C, 1], F32, isOutput=False),
        "selg": nc.declare_dram_parameter("selg", [128, 128], F32R, isOutput=False),
        "out": nc.declare_dram_parameter("out", [BS, C, N], F32, isOutput=True),
    }
    with tile.TileContext(nc) as tc:
        with ExitStack() as ctx:
            _build_tile(ctx, tc, te)
    nc.finalize()
    return nc


def _make_selg() -> np.ndarray:
    # block-diagonal group-average matrix: selg[i, j] = 1/16 if same 16-ch group
    selg = np.zeros((128, 128), np.float32)
    for j in range(128):
        g0 = (j // 16) * 16
        selg[g0 : g0 + 16, j] = 1.0 / 16.0
    return selg


def make_in_maps(inputs: dict) -> list[dict]:
    x = np.ascontiguousarray(np.asarray(inputs["x"], np.float32)).reshape(B, C, N)
    w_qkv = np.asarray(inputs["w_qkv"], np.float32)
    b_qkv = np.asarray(inputs["b_qkv"], np.float32)
    w_proj = np.asarray(inputs["w_proj"], np.float32)
    b_proj = np.asarray(inputs["b_proj"], np.float32)
    gamma = np.asarray(inputs["gamma"], np.float32)
    beta = np.asarray(inputs["beta"], np.float32)

    bf = ml_dtypes.bfloat16
    common = {
        "wqkT": np.ascontiguousarray(w_qkv[: 2 * C, :].T).astype(bf),
        "wvT": np.ascontiguousarray(w_qkv[2 * C :, :].T).astype(bf),
        "wpT": np.ascontiguousarray(w_proj.T).astype(bf),
        "bq": (b_qkv[:C] * float(D) ** -0.5).reshape(C, 1).astype(np.float32),
        "beff": (b_proj + w_proj @ b_qkv[2 * C :]).reshape(C, 1).astype(np.float32),
        "gamma": gamma.reshape(C, 1).copy(),
        "beta": beta.reshape(C, 1).copy(),
        "selg": _make_selg(),
    }
    x_bf = x.astype(bf)
    return [
        {"x": np.ascontiguousarray(x_bf[i * BS : (i + 1) * BS]), **common}
        for i in range(NCORES)
    ]


def kernel(**inputs) -> np.ndarray:
    global LAST_EXEC_NS, LAST_RESULTS
    nc = build_bass()
    in_maps = make_in_maps(inputs)
    res = run_bass_kernel_spmd(nc, in_maps, list(range(NCORES)))
    LAST_RESULTS = res
    LAST_EXEC_NS = res.exec_time_ns
    out = np.concatenate([np.asarray(res.results[i]["out"]) for i in range(NCORES)], 0)
    return out.reshape(B, C, H, W).astype(np.float32)
